# revision 21
# baseline (speedup 1.0000x reference)
"""GAT (2-layer graph attention network) Trainium2 kernel, 8-core SPMD.

Problem: nn_GAT_22127671509494
  N=4096 nodes, F_IN=512, F_HID=64, H=8 heads, C=40 classes, lrelu slope 0.2.

Sharding: 8 cores = 2 head-groups x 4 query-slices.
  core c: g = c//4 (heads g*4..g*4+4), s = c%4 (query rows s*1024..(s+1)*1024).
  Layer 1: each core computes h (layer-1 output, transposed: [256 feats, 1024 q])
  for its (head-group, q-slice). AllGather over all 8 cores -> full hT [512, 4096].
  Layer 2 is recomputed per q-slice (duplicated across the 2 head-groups;
  host takes the g=0 copies).

Key math trick: with binary mask adj and s = f1[i]+f2[j],
  exp(leaky_relu(s)) = max(exp(s), exp(0.2 s))
                     = exp(0.2 f1[i]) * exp(0.2 f2[j]) * max(w[i] z[j], 1),
  w = exp(0.8 f1), z = exp(0.8 f2).
The exp(0.2 f1[i]) factor cancels in the softmax; exp(0.2 f2[j]) is folded
into the matmul lhsT (scaling Wh rows + the appended ones column). So the
NxN score tile needs NO transcendentals: one tensor_scalar (mult,max) and one
tensor_tensor (mask multiply) per tile. Softmax denominator comes free via the
ones column appended to Wh in the attention matmul.
"""

import numpy as np
import ml_dtypes

import concourse.bacc as bacc
import concourse.bass as bass
import concourse.tile as tile
import concourse.mybir as mybir
from concourse.bass_utils import run_bass_kernel_spmd

F32 = mybir.dt.float32
F32R = mybir.dt.float32r
BF16 = mybir.dt.bfloat16
FP8 = mybir.dt.float8e4
AF = mybir.ActivationFunctionType
ALU = mybir.AluOpType
AX = mybir.AxisListType

N = 4096
FIN = 512
FH = 64
H = 8
C = 40
NCORES = 8
NG = 2          # head groups
NS = 4          # query slices
HG = H // NG    # heads per group (4)
QL = N // NS    # queries per core, layer 1 (1024)
NT = N // 128   # target-node tiles (32)
KT = FIN // 128  # contraction chunks (4)

ADJ_DT = BF16
ADJ_NP = ml_dtypes.bfloat16
SC_DT = BF16  # score-pipeline dtype

_CACHE = {}


def _build(dbg=False):
    nc = bacc.Bacc("TRN2", target_bir_lowering=False, debug=False)

    # ---- I/O ----
    xT = nc.dram_tensor("xT", [FIN, N], BF16, kind="ExternalInput")
    xTq = nc.dram_tensor("xTq", [FIN, QL], BF16, kind="ExternalInput")
    adjT = nc.dram_tensor("adjT", [N, QL], ADJ_DT, kind="ExternalInput")
    Wa = nc.dram_tensor("Wa", [FIN, HG * 66], BF16, kind="ExternalInput")
    W1R = nc.dram_tensor("W1R", [FIN, HG * 128], BF16, kind="ExternalInput")
    Wo = nc.dram_tensor("Wo", [FIN, 42], BF16, kind="ExternalInput")
    Wo1Rg = nc.dram_tensor("Wo1Rg", [256, 128], BF16, kind="ExternalInput")
    WSel = nc.dram_tensor("WSel", [NCORES, 128], BF16, kind="ExternalInput")
    out = nc.dram_tensor("out", [QL, C], F32, kind="ExternalOutput")

    # collective bounce buffers (payload rows: 256 hT feats + 1 f1_2 partial)
    ag_in = nc.dram_tensor("ag_in", [257, QL], BF16)
    ag_out = nc.dram_tensor("ag_out", [NCORES, 257, QL], BF16, addr_space="Shared")

    dbgt = {}
    if dbg:
        for nm, shp in [("d_wb", [128, HG * QL]),
                        ("d_zv", [128, HG * NT]), ("d_v2", [128, HG * NT]),
                        ("d_htl", [128, 2 * QL]), ("d_agr", [NCORES, QL]),
                        ("d_w2b", [128, QL]), ("d_wh2", [128, NT * 42]),
                        ("d_q0", [128, QL]), ("d_acc0", [65, 512]),
                        ("d_rz", [1, 512]), ("d_zb", [64, 512]), ("d_xv", [64, 512]),
                        ("d_mv", [64, 512]), ("d_ev", [64, 512]), ("d_rv", [64, 512])]:
            dbgt[nm] = nc.dram_tensor(nm, shp, F32, kind="ExternalOutput")
    with tile.TileContext(nc) as tc:
        _emit(nc, tc, xT, xTq, adjT, Wa, W1R, Wo, Wo1Rg, WSel, out, ag_in, ag_out, dbgt)
    nc.compile()
    return nc


def _emit(nc, tc, xT, xTq, adjT, Wa, W1R, Wo, Wo1Rg, WSel, out, ag_in, ag_out, dbgt={}):
    import contextlib

    est = contextlib.ExitStack()
    with est:
        const = est.enter_context(tc.tile_pool(name="const", bufs=1))
        adjp = est.enter_context(tc.tile_pool(name="adjp", bufs=1))
        htlp = est.enter_context(tc.tile_pool(name="htlp", bufs=1))
        smalls = est.enter_context(tc.tile_pool(name="smalls", bufs=2))

        # ---- constant loads ----
        Wa_sb = const.tile([128, KT, HG * 66], BF16)
        W1R_sb = const.tile([128, KT, HG * 128], BF16)
        Wo_sb = const.tile([128, KT, 42], BF16)
        Wo1Rg_sb = const.tile([128, 2, 128], BF16)
        WSel_sb = const.tile([NCORES, 128], BF16)
        xTq_sb = const.tile([128, KT, QL], BF16)
        for kt in range(KT):
            nc.sync.dma_start(out=Wa_sb[:, kt, :], in_=Wa[kt * 128:(kt + 1) * 128, :])
            nc.sync.dma_start(out=W1R_sb[:, kt, :], in_=W1R[kt * 128:(kt + 1) * 128, :])
            nc.sync.dma_start(out=Wo_sb[:, kt, :], in_=Wo[kt * 128:(kt + 1) * 128, :])
            nc.sync.dma_start(out=xTq_sb[:, kt, :], in_=xTq[kt * 128:(kt + 1) * 128, :])
        for t in range(2):
            nc.sync.dma_start(out=Wo1Rg_sb[:, t, :], in_=Wo1Rg[t * 128:(t + 1) * 128, :])
        nc.sync.dma_start(out=WSel_sb[:, :], in_=WSel[:, :])

        neg1 = const.tile([128, 1], F32)
        nc.vector.memset(neg1[:], -1.0)

        # adjacency (transposed, fp8), resident for both layers
        adj_sb = adjp.tile([128, NT, QL], ADJ_DT)
        for t in range(NT):
            nc.scalar.dma_start(out=adj_sb[:, t, :], in_=adjT[t * 128:(t + 1) * 128, :])

        # layer-1 local output, transposed: [128, t, ql] covering 256 feats
        hTl = htlp.tile([128, 2, QL], BF16)
        prow = htlp.tile([1, QL], BF16)

        with contextlib.ExitStack() as l1:
            whp = l1.enter_context(tc.tile_pool(name="whp", bufs=1))
            xa = l1.enter_context(tc.tile_pool(name="xa", bufs=6))
            wbp = l1.enter_context(tc.tile_pool(name="wbp", bufs=1))
            sA = l1.enter_context(tc.tile_pool(name="sA", bufs=2))
            sQ = l1.enter_context(tc.tile_pool(name="sQ", bufs=2))
            sW = l1.enter_context(tc.tile_pool(name="sW", bufs=3))
            ps_a = l1.enter_context(tc.tile_pool(name="ps_a", bufs=2, space="PSUM"))
            ps_f1 = l1.enter_context(tc.tile_pool(name="ps_f1", bufs=1, space="PSUM"))
            ps_acc = l1.enter_context(tc.tile_pool(name="ps_acc", bufs=4, space="PSUM"))

            # ---- stage A: WhAug[n, :] = x @ Wa  (node-major, all 4096 nodes) ----
            wh_sb = whp.tile([128, NT, HG * 66], BF16)
            for q4 in range(4):
                xts = [xa.tile([128, QL], BF16, tag="xa", name=f"xa_{q4}_{kt}")
                       for kt in range(KT)]
                for kt in range(KT):
                    nc.sync.dma_start(
                        out=xts[kt][:],
                        in_=xT[kt * 128:(kt + 1) * 128,
                               q4 * QL:(q4 + 1) * QL],
                    )
                for mq in range(8):
                    mt = q4 * 8 + mq
                    acc = ps_a.tile([128, HG * 66], F32)
                    for kt in range(KT):
                        nc.tensor.matmul(
                            acc[:], xts[kt][:, mq * 128:(mq + 1) * 128], Wa_sb[:, kt, :],
                            start=(kt == 0), stop=(kt == KT - 1),
                        )
                    nc.scalar.activation(wh_sb[:, mt, :], acc[:], AF.Copy)
            # ones columns (col 65 of each head block)
            nc.vector.memset(wh_sb[:, :, 65::66], 1.0)

            # per-head f2-derived columns: z = exp(0.8 f2), v2 = exp(0.2 f2)
            zv = wbp.tile([128, HG, NT], F32)
            v2 = wbp.tile([128, HG, NT], F32)
            w_b = wbp.tile([128, HG, QL], SC_DT)
            for j in range(HG):
                nc.scalar.activation(zv[:, j, :], wh_sb[:, :, j * 66], AF.Exp, scale=0.8)
                nc.scalar.activation(v2[:, j, :], wh_sb[:, :, j * 66], AF.Exp, scale=0.2)
                # w broadcast tile: W1R.T @ xTq -> f1[q] on every partition
                f1p = ps_f1.tile([128, QL], F32)
                for kt in range(KT):
                    for qh in range(2):
                        nc.tensor.matmul(
                            f1p[:, qh * 512:(qh + 1) * 512],
                            W1R_sb[:, kt, j * 128:(j + 1) * 128],
                            xTq_sb[:, kt, qh * 512:(qh + 1) * 512],
                            start=(kt == 0), stop=(kt == KT - 1),
                        )
                nc.scalar.activation(w_b[:, j, :], f1p[:], AF.Exp, scale=0.8)

            # ---- stage B: attention per head ----
            for j in range(HG):
                accs = [ps_acc.tile([65, 512], F32, tag="acc", name=f"acc_{j}_{i}") for i in range(2)]
                for t in range(NT):
                    gi = j * NT + t
                    mode_act = (gi % 2 == 0)
                    p2_gp = (gi % 8 in (1, 5))
                    whS = sW.tile([128, 65], SC_DT, tag="whS")
                    nc.gpsimd.tensor_scalar_mul(
                        whS[:], wh_sb[:, t, j * 66 + 1: j * 66 + 66], v2[:, j, t:t + 1]
                    )
                    a_t = sA.tile([128, QL], SC_DT, tag="sa")
                    if mode_act:
                        # A' = relu(w*z - 1) = max(w*z, 1) - 1; the missing
                        # "+1" rides as an extra adjT matmul below.
                        nc.scalar.activation(
                            a_t[:], w_b[:, j, :], AF.Relu,
                            bias=neg1[:], scale=zv[:, j, t:t + 1],
                        )
                    else:
                        nc.vector.tensor_scalar(
                            a_t[:], w_b[:, j, :], zv[:, j, t:t + 1], 1.0,
                            ALU.mult, ALU.max,
                        )
                    q_t = sQ.tile([128, QL], SC_DT, tag="sq")
                    eng = nc.gpsimd if p2_gp else nc.vector
                    eng.tensor_tensor(q_t[:], a_t[:], adj_sb[:, t, :], ALU.mult)

                    for qh in range(2):
                        nc.tensor.matmul(
                            accs[qh][:], whS[:], q_t[:, qh * 512:(qh + 1) * 512],
                            start=(t == 0),
                            stop=(t == NT - 1 and not mode_act),
                        )
                        if mode_act:
                            nc.tensor.matmul(
                                accs[qh][:], whS[:],
                                adj_sb[:, t, qh * 512:(qh + 1) * 512],
                                start=False, stop=(t == NT - 1),
                            )
                # epilogue: divide by Z (row 64), elu, write into hTl
                for qh in range(2):
                    acc = accs[qh]
                    acc_sb = smalls.tile([65, 512], F32, tag="acc_sb")
                    nc.scalar.activation(acc_sb[:], acc[:], AF.Copy)
                    if dbgt and j == 0 and qh == 0:
                        nc.sync.dma_start(out=dbgt["d_acc0"][:, :], in_=acc_sb[:])
                    zrow = smalls.tile([1, 512], F32, tag="zrow")
                    nc.sync.dma_start(out=zrow[:], in_=acc_sb[64:65, :])
                    rz = smalls.tile([1, 512], F32, tag="rz")
                    nc.vector.reciprocal_approx_fast(rz[:], zrow[:])
                    zb = smalls.tile([64, 512], F32, tag="zb")
                    nc.gpsimd.partition_broadcast(zb[:], rz[:])
                    xv = smalls.tile([64, 512], F32, tag="xv")
                    nc.vector.tensor_tensor(xv[:], acc_sb[0:64, :], zb[:], ALU.mult)
                    if dbgt and j == 0 and qh == 0:
                        nc.sync.dma_start(out=dbgt["d_rz"][:, :], in_=rz[:])
                        nc.sync.dma_start(out=dbgt["d_zb"][:, :], in_=zb[:])
                        nc.sync.dma_start(out=dbgt["d_xv"][:, :], in_=xv[:])
                    mv = smalls.tile([64, 512], F32, tag="mv")
                    nc.vector.tensor_scalar_min(mv[:], xv[:], 0.0)
                    ev = smalls.tile([64, 512], F32, tag="ev")
                    nc.scalar.activation(ev[:], mv[:], AF.Exp)
                    rv = smalls.tile([64, 512], F32, tag="rv")
                    nc.vector.tensor_sub(rv[:], xv[:], mv[:])
                    if dbgt and j == 0 and qh == 0:
                        nc.sync.dma_start(out=dbgt["d_mv"][:, :], in_=mv[:])
                        nc.sync.dma_start(out=dbgt["d_ev"][:, :], in_=ev[:])
                        nc.sync.dma_start(out=dbgt["d_rv"][:, :], in_=rv[:])
                    hv = smalls.tile([64, 512], BF16, tag="hv")
                    nc.vector.affine_then_add(hv[:], rv[:], ev[:], scale=1.0, bias=-1.0)
                    nc.sync.dma_start(
                        out=hTl[(j % 2) * 64:(j % 2) * 64 + 64, j // 2,
                                qh * 512:(qh + 1) * 512],
                        in_=hv[:],
                    )

            if dbgt:
                nc.sync.dma_start(out=dbgt["d_zv"][:, :], in_=zv[:].rearrange("p a b -> p (a b)"))
                nc.sync.dma_start(out=dbgt["d_v2"][:, :], in_=v2[:].rearrange("p a b -> p (a b)"))

            # partial f1_2 (our feature block): row = w1out_g . hTl
            f12p = ps_f1.tile([128, QL], F32, tag="f1p", name="f12p")
            for t in range(2):
                for qh in range(2):
                    nc.tensor.matmul(
                        f12p[:, qh * 512:(qh + 1) * 512],
                        Wo1Rg_sb[:, t, :],
                        hTl[:, t, qh * 512:(qh + 1) * 512],
                        start=(t == 0), stop=(t == 1),
                    )
            nc.scalar.activation(prow[:], f12p[0:1, :], AF.Copy)

        # ---- stage C: AllGather hT (+ f1_2 partial row) ----
        nc.sync.dma_start(
            out=ag_in[0:256, :].rearrange("(t p) q -> p t q", p=128), in_=hTl[:]
        )
        nc.sync.dma_start(out=ag_in[256:257, :], in_=prow[:])
        nc.gpsimd.collective_compute(
            "AllGather",
            ALU.bypass,
            ins=[ag_in[:, :].opt()],
            outs=[ag_out[:, :, :].opt()],
            replica_groups=[list(range(NCORES))],
        )

        # ---- stage D: layer 2 ----
        wh2p = est.enter_context(tc.tile_pool(name="wh2p", bufs=1))
        w2bp = est.enter_context(tc.tile_pool(name="w2bp", bufs=1))
        with contextlib.ExitStack() as l2a:
            htag = l2a.enter_context(tc.tile_pool(name="htag", bufs=4))
            ps_w2 = l2a.enter_context(tc.tile_pool(name="ps_w2", bufs=2, space="PSUM"))
            ps_f2 = l2a.enter_context(tc.tile_pool(name="ps_f2", bufs=1, space="PSUM"))

            wh2_sb = wh2p.tile([128, NT, 42], F32)
            w2_b = w2bp.tile([128, QL], SC_DT)
            z2 = w2bp.tile([128, NT], F32)
            v22 = w2bp.tile([128, NT], F32)

            # Wh2Aug = h @ Wo  (node-major over all 4096 nodes)
            for sr in range(NS):
                hts = [htag.tile([128, QL], BF16, tag="htag", name=f"htag_{sr}_{i}") for i in range(KT)]
                for kf in range(KT):
                    nc.scalar.dma_start(
                        out=hts[kf][:],
                        in_=ag_out[(kf // 2) * 4 + sr,
                                   (kf % 2) * 128:(kf % 2) * 128 + 128, :],
                    )
                for mq in range(8):
                    mt = sr * 8 + mq
                    acc = ps_w2.tile([128, 42], F32)
                    for kf in range(KT):
                        nc.tensor.matmul(
                            acc[:], hts[kf][:, mq * 128:(mq + 1) * 128], Wo_sb[:, kf, :],
                            start=(kf == 0), stop=(kf == KT - 1),
                        )
                    nc.scalar.activation(wh2_sb[:, mt, :], acc[:], AF.Copy)
            nc.vector.memset(wh2_sb[:, :, 41:42], 1.0)
            nc.scalar.activation(z2[:], wh2_sb[:, :, 0], AF.Exp, scale=0.8)
            nc.scalar.activation(v22[:], wh2_sb[:, :, 0], AF.Exp, scale=0.2)

            # f1_2 for our q-slice: sum the two partner partial rows via WSel
            agr = w2bp.tile([NCORES, QL], BF16)
            nc.sync.dma_start(out=agr[:], in_=ag_out[:, 256, :])
            f2p = ps_f2.tile([128, QL], F32)
            for qh in range(2):
                nc.tensor.matmul(
                    f2p[:, qh * 512:(qh + 1) * 512],
                    WSel_sb[:],
                    agr[:, qh * 512:(qh + 1) * 512],
                    start=True, stop=True,
                )
            nc.scalar.activation(w2_b[:], f2p[:], AF.Exp, scale=0.8)
            if dbgt:
                nc.sync.dma_start(out=dbgt["d_wh2"][:, :], in_=wh2_sb[:].rearrange("p a b -> p (a b)"))

        with contextlib.ExitStack() as l2b:
            sA2 = l2b.enter_context(tc.tile_pool(name="sA2", bufs=2))
            sQ2 = l2b.enter_context(tc.tile_pool(name="sQ2", bufs=2))
            sW2 = l2b.enter_context(tc.tile_pool(name="sW2", bufs=3))
            ps_o2 = l2b.enter_context(tc.tile_pool(name="ps_o2", bufs=8, space="PSUM"))

            o2 = [ps_o2.tile([128, 42], F32, tag="o2", name=f"o2_{i}") for i in range(8)]
            for t in range(NT):
                wh2S = sW2.tile([128, 42], SC_DT, tag="wh2S")
                nc.vector.tensor_scalar_mul(wh2S[:], wh2_sb[:, t, :], v22[:, t:t + 1])
                a2 = sA2.tile([128, QL], SC_DT, tag="sa2")
                nc.vector.tensor_scalar(
                    a2[:], w2_b[:], z2[:, t:t + 1], 1.0, ALU.mult, ALU.max
                )
                q2 = sQ2.tile([128, QL], SC_DT, tag="sq2")
                eng = nc.gpsimd if (t % 4 == 3) else nc.vector
                eng.tensor_tensor(q2[:], a2[:], adj_sb[:, t, :], ALU.mult)
                for qi in range(8):
                    nc.tensor.matmul(
                        o2[qi][:], q2[:, qi * 128:(qi + 1) * 128], wh2S[:],
                        start=(t == 0), stop=(t == NT - 1),
                    )
            # epilogue: divide, elu, log_softmax (Exp phase, then Ln phase,
            # so the ACT table set switches at most once)
            keep = l2b.enter_context(tc.tile_pool(name="keep", bufs=8))
            h2s, nmxs, sss = [], [], []
            for qi in range(8):
                acc = o2[qi]
                rz = smalls.tile([128, 1], F32, tag="rz2")
                nc.vector.reciprocal(rz[:], acc[:, 41:42])
                x2 = smalls.tile([128, C], F32, tag="x2")
                nc.vector.tensor_scalar_mul(x2[:], acc[:, 1:41], rz[:])
                m2 = smalls.tile([128, C], F32, tag="m2")
                nc.vector.tensor_scalar_min(m2[:], x2[:], 0.0)
                e2 = smalls.tile([128, C], F32, tag="e2")
                nc.scalar.activation(e2[:], m2[:], AF.Exp)
                r2 = smalls.tile([128, C], F32, tag="r2")
                nc.vector.tensor_sub(r2[:], x2[:], m2[:])
                h2 = keep.tile([128, C], F32, tag="h2", name=f"h2_{qi}")
                nc.vector.affine_then_add(h2[:], r2[:], e2[:], scale=1.0, bias=-1.0)
                nmx = keep.tile([128, 1], F32, tag="nmx", name=f"nmx_{qi}")
                nc.vector.tensor_reduce(nmx[:], h2[:], AX.X, ALU.max, negate=True)
                es = smalls.tile([128, C], F32, tag="es")
                ss = keep.tile([128, 1], F32, tag="ss", name=f"ss_{qi}")
                nc.scalar.activation(es[:], h2[:], AF.Exp, bias=nmx[:], accum_out=ss[:])
                h2s.append(h2); nmxs.append(nmx); sss.append(ss)
            for qi in range(8):
                ls = smalls.tile([128, 1], F32, tag="ls")
                nc.scalar.activation(ls[:], sss[qi][:], AF.Ln)
                res = smalls.tile([128, C], F32, tag="res")
                nc.vector.tensor_scalar(
                    res[:], h2s[qi][:], nmxs[qi][:], ls[:], ALU.add, ALU.subtract
                )
                nc.sync.dma_start(out=out[qi * 128:(qi + 1) * 128, :], in_=res[:])


def _marshal(x, adj, W_heads, a_heads, W_out, a_out):
    xTf = np.ascontiguousarray(x.T)
    w1out = (W_out @ a_out[:C]).astype(np.float32)          # [512]
    w2out = (W_out @ a_out[C:]).astype(np.float32)          # [512]
    Wo_m = np.zeros((FIN, 42), np.float32)
    Wo_m[:, 0] = w2out
    Wo_m[:, 1:41] = W_out
    in_maps = []
    for c in range(NCORES):
        g, s = c // NS, c % NS
        heads = range(g * HG, (g + 1) * HG)
        Wa_m = np.zeros((FIN, HG * 66), np.float32)
        W1R_m = np.zeros((FIN, HG * 128), np.float32)
        for jj, h in enumerate(heads):
            Wa_m[:, jj * 66] = W_heads[h] @ a_heads[h, FH:]
            Wa_m[:, jj * 66 + 1: jj * 66 + 65] = W_heads[h]
            W1R_m[:, jj * 128:(jj + 1) * 128] = (W_heads[h] @ a_heads[h, :FH])[:, None]
        WSel_m = np.zeros((NCORES, 128), np.float32)
        for r in range(NCORES):
            if r % NS == s:
                WSel_m[r, :] = 1.0
        in_maps.append({
            "xT": xTf.astype(ml_dtypes.bfloat16),
            "xTq": np.ascontiguousarray(xTf[:, s * QL:(s + 1) * QL]).astype(ml_dtypes.bfloat16),
            "adjT": np.ascontiguousarray(adj[s * QL:(s + 1) * QL, :].T).astype(ADJ_NP),
            "Wa": Wa_m.astype(ml_dtypes.bfloat16),
            "W1R": W1R_m.astype(ml_dtypes.bfloat16),
            "Wo": Wo_m.astype(ml_dtypes.bfloat16),
            "Wo1Rg": np.ascontiguousarray(
                np.broadcast_to(w1out[g * 256:(g + 1) * 256, None], (256, 128))
            ).astype(ml_dtypes.bfloat16),
            "WSel": WSel_m.astype(ml_dtypes.bfloat16),
        })
    return in_maps


def kernel(**inputs):
    x = np.asarray(inputs["x"], np.float32)
    adj = np.asarray(inputs["adj"], np.float32)
    W_heads = np.asarray(inputs["W_heads"], np.float32)
    a_heads = np.asarray(inputs["a_heads"], np.float32)
    W_out = np.asarray(inputs["W_out"], np.float32)
    a_out = np.asarray(inputs["a_out"], np.float32)

    if "nc" not in _CACHE:
        _CACHE["nc"] = _build()
    nc = _CACHE["nc"]
    in_maps = _marshal(x, adj, W_heads, a_heads, W_out, a_out)
    res = run_bass_kernel_spmd(nc, in_maps, core_ids=list(range(NCORES)))
    full = np.empty((N, C), np.float32)
    for c in range(NS):  # g=0 copies carry the results
        full[c * QL:(c + 1) * QL] = res.results[c]["out"]
    return full


# revision 22
# speedup vs baseline: 12.8774x; 12.8774x over previous
"""GAT (2-layer graph attention network) Trainium2 kernel, 8-core SPMD.

Problem: nn_GAT_22127671509494
  N=4096 nodes, F_IN=512, F_HID=64, H=8 heads, C=40 classes, lrelu slope 0.2.

Sharding: 8 cores = 2 head-groups x 4 query-slices.
  core c: g = c//4 (heads g*4..g*4+4), s = c%4 (query rows s*1024..(s+1)*1024).
  Layer 1: each core computes h (layer-1 output, transposed: [256 feats, 1024 q])
  for its (head-group, q-slice). AllGather over all 8 cores -> full hT [512, 4096].
  Layer 2 is recomputed per q-slice (duplicated across the 2 head-groups;
  host takes the g=0 copies).

Key math trick: with binary mask adj and s = f1[i]+f2[j],
  exp(leaky_relu(s)) = max(exp(s), exp(0.2 s))
                     = exp(0.2 f1[i]) * exp(0.2 f2[j]) * max(w[i] z[j], 1),
  w = exp(0.8 f1), z = exp(0.8 f2).
The exp(0.2 f1[i]) factor cancels in the softmax; exp(0.2 f2[j]) is folded
into the matmul lhsT (scaling Wh rows + the appended ones column). So the
NxN score tile needs NO transcendentals: one tensor_scalar (mult,max) and one
tensor_tensor (mask multiply) per tile. Softmax denominator comes free via the
ones column appended to Wh in the attention matmul.
"""

import numpy as np
import ml_dtypes

import concourse.bacc as bacc
import concourse.bass as bass
import concourse.tile as tile
import concourse.mybir as mybir
from concourse.bass_utils import run_bass_kernel_spmd

F32 = mybir.dt.float32
F32R = mybir.dt.float32r
BF16 = mybir.dt.bfloat16
FP8 = mybir.dt.float8e4
AF = mybir.ActivationFunctionType
ALU = mybir.AluOpType
AX = mybir.AxisListType

N = 4096
FIN = 512
FH = 64
H = 8
C = 40
NCORES = 8
NG = 2          # head groups
NS = 4          # query slices
HG = H // NG    # heads per group (4)
QL = N // NS    # queries per core, layer 1 (1024)
NT = N // 128   # target-node tiles (32)
KT = FIN // 128  # contraction chunks (4)

ADJ_DT = BF16
ADJ_NP = ml_dtypes.bfloat16
SC_DT = BF16  # score-pipeline dtype

_CACHE = {}


def _build(dbg=False, reps=1):
    nc = bacc.Bacc("TRN2", target_bir_lowering=False, debug=False)

    # ---- I/O ----
    xT = nc.dram_tensor("xT", [FIN, N], BF16, kind="ExternalInput")
    xTq = nc.dram_tensor("xTq", [FIN, QL], BF16, kind="ExternalInput")
    adjT = nc.dram_tensor("adjT", [N, QL], ADJ_DT, kind="ExternalInput")
    Wa = nc.dram_tensor("Wa", [FIN, HG * 66], BF16, kind="ExternalInput")
    W1R = nc.dram_tensor("W1R", [FIN, HG * 128], BF16, kind="ExternalInput")
    Wo = nc.dram_tensor("Wo", [FIN, 42], BF16, kind="ExternalInput")
    Wo1Rg = nc.dram_tensor("Wo1Rg", [256, 128], BF16, kind="ExternalInput")
    WSel = nc.dram_tensor("WSel", [NCORES, 128], BF16, kind="ExternalInput")
    out = nc.dram_tensor("out", [QL, C], F32, kind="ExternalOutput")

    # collective bounce buffers (payload rows: 256 hT feats + 1 f1_2 partial)
    ag_in = nc.dram_tensor("ag_in", [257, QL], BF16)
    ag_out = nc.dram_tensor("ag_out", [NCORES, 257, QL], BF16, addr_space="Shared")

    dbgt = {}
    if dbg:
        for nm, shp in [("d_wb", [128, HG * QL]),
                        ("d_zv", [128, HG * NT]), ("d_v2", [128, HG * NT]),
                        ("d_htl", [128, 2 * QL]), ("d_agr", [NCORES, QL]),
                        ("d_w2b", [128, QL]), ("d_wh2", [128, NT * 42]),
                        ("d_q0", [128, QL]), ("d_acc0", [65, 512]),
                        ("d_rz", [1, 512]), ("d_zb", [64, 512]), ("d_xv", [64, 512]),
                        ("d_mv", [64, 512]), ("d_ev", [64, 512]), ("d_rv", [64, 512])]:
            dbgt[nm] = nc.dram_tensor(nm, shp, F32, kind="ExternalOutput")
    with tile.TileContext(nc) as tc:
        for _rep in range(reps):
            _emit(nc, tc, xT, xTq, adjT, Wa, W1R, Wo, Wo1Rg, WSel, out, ag_in, ag_out,
                  dbgt if _rep == 0 else {})
    nc.compile()
    return nc


def _emit(nc, tc, xT, xTq, adjT, Wa, W1R, Wo, Wo1Rg, WSel, out, ag_in, ag_out, dbgt={}):
    import contextlib

    est = contextlib.ExitStack()
    with est:
        const = est.enter_context(tc.tile_pool(name="const", bufs=1))
        adjp = est.enter_context(tc.tile_pool(name="adjp", bufs=1))
        htlp = est.enter_context(tc.tile_pool(name="htlp", bufs=1))
        smalls = est.enter_context(tc.tile_pool(name="smalls", bufs=2))

        # ---- constant loads ----
        Wa_sb = const.tile([128, KT, HG * 66], BF16)
        W1R_sb = const.tile([128, KT, HG * 128], BF16)
        Wo_sb = const.tile([128, KT, 42], BF16)
        Wo1Rg_sb = const.tile([128, 2, 128], BF16)
        WSel_sb = const.tile([NCORES, 128], BF16)
        xTq_sb = const.tile([128, KT, QL], BF16)
        for kt in range(KT):
            nc.sync.dma_start(out=Wa_sb[:, kt, :], in_=Wa[kt * 128:(kt + 1) * 128, :])
            nc.sync.dma_start(out=W1R_sb[:, kt, :], in_=W1R[kt * 128:(kt + 1) * 128, :])
            nc.sync.dma_start(out=Wo_sb[:, kt, :], in_=Wo[kt * 128:(kt + 1) * 128, :])
            nc.sync.dma_start(out=xTq_sb[:, kt, :], in_=xTq[kt * 128:(kt + 1) * 128, :])
        for t in range(2):
            nc.sync.dma_start(out=Wo1Rg_sb[:, t, :], in_=Wo1Rg[t * 128:(t + 1) * 128, :])
        nc.sync.dma_start(out=WSel_sb[:, :], in_=WSel[:, :])

        neg1 = const.tile([128, 1], F32)
        nc.vector.memset(neg1[:], -1.0)

        # adjacency (transposed, fp8), resident for both layers
        adj_sb = adjp.tile([128, NT, QL], ADJ_DT)
        for t in range(NT):
            nc.scalar.dma_start(out=adj_sb[:, t, :], in_=adjT[t * 128:(t + 1) * 128, :])

        # layer-1 local output, transposed: [128, t, ql] covering 256 feats
        hTl = htlp.tile([128, 2, QL], BF16)
        prow = htlp.tile([1, QL], BF16)

        with contextlib.ExitStack() as l1:
            whp = l1.enter_context(tc.tile_pool(name="whp", bufs=1))
            xa = l1.enter_context(tc.tile_pool(name="xa", bufs=6))
            wbp = l1.enter_context(tc.tile_pool(name="wbp", bufs=1))
            sA = l1.enter_context(tc.tile_pool(name="sA", bufs=2))
            sQ = l1.enter_context(tc.tile_pool(name="sQ", bufs=2))
            sW = l1.enter_context(tc.tile_pool(name="sW", bufs=3))
            ps_a = l1.enter_context(tc.tile_pool(name="ps_a", bufs=2, space="PSUM"))
            ps_f1 = l1.enter_context(tc.tile_pool(name="ps_f1", bufs=1, space="PSUM"))
            ps_acc = l1.enter_context(tc.tile_pool(name="ps_acc", bufs=4, space="PSUM"))

            # ---- stage A: WhAug[n, :] = x @ Wa  (node-major, all 4096 nodes) ----
            wh_sb = whp.tile([128, NT, HG * 66], BF16)
            for q4 in range(4):
                xts = [xa.tile([128, QL], BF16, tag="xa", name=f"xa_{q4}_{kt}")
                       for kt in range(KT)]
                for kt in range(KT):
                    nc.sync.dma_start(
                        out=xts[kt][:],
                        in_=xT[kt * 128:(kt + 1) * 128,
                               q4 * QL:(q4 + 1) * QL],
                    )
                for mq in range(8):
                    mt = q4 * 8 + mq
                    acc = ps_a.tile([128, HG * 66], F32)
                    for kt in range(KT):
                        nc.tensor.matmul(
                            acc[:], xts[kt][:, mq * 128:(mq + 1) * 128], Wa_sb[:, kt, :],
                            start=(kt == 0), stop=(kt == KT - 1),
                        )
                    nc.scalar.activation(wh_sb[:, mt, :], acc[:], AF.Copy)
            # ones columns (col 65 of each head block)
            nc.vector.memset(wh_sb[:, :, 65::66], 1.0)

            # per-head f2-derived columns: z = exp(0.8 f2), v2 = exp(0.2 f2)
            zv = wbp.tile([128, HG, NT], F32)
            v2 = wbp.tile([128, HG, NT], F32)
            w_b = wbp.tile([128, HG, QL], SC_DT)
            for j in range(HG):
                nc.scalar.activation(zv[:, j, :], wh_sb[:, :, j * 66], AF.Exp, scale=0.8)
                nc.scalar.activation(v2[:, j, :], wh_sb[:, :, j * 66], AF.Exp, scale=0.2)
                # w broadcast tile: W1R.T @ xTq -> f1[q] on every partition
                f1p = ps_f1.tile([128, QL], F32)
                for kt in range(KT):
                    for qh in range(2):
                        nc.tensor.matmul(
                            f1p[:, qh * 512:(qh + 1) * 512],
                            W1R_sb[:, kt, j * 128:(j + 1) * 128],
                            xTq_sb[:, kt, qh * 512:(qh + 1) * 512],
                            start=(kt == 0), stop=(kt == KT - 1),
                        )
                nc.scalar.activation(w_b[:, j, :], f1p[:], AF.Exp, scale=0.8)

            # ---- stage B: attention per head ----
            for j in range(HG):
                accs = [ps_acc.tile([65, 512], F32, tag="acc", name=f"acc_{j}_{i}") for i in range(2)]
                for t in range(NT):
                    gi = j * NT + t
                    mode_act = (gi % 2 == 0)
                    p2_gp = (gi % 8 in (1, 5))
                    whS = sW.tile([128, 65], SC_DT, tag="whS")
                    nc.gpsimd.tensor_scalar_mul(
                        whS[:], wh_sb[:, t, j * 66 + 1: j * 66 + 66], v2[:, j, t:t + 1]
                    )
                    a_t = sA.tile([128, QL], SC_DT, tag="sa")
                    if mode_act:
                        # A' = relu(w*z - 1) = max(w*z, 1) - 1; the missing
                        # "+1" rides as an extra adjT matmul below.
                        nc.scalar.activation(
                            a_t[:], w_b[:, j, :], AF.Relu,
                            bias=neg1[:], scale=zv[:, j, t:t + 1],
                        )
                    else:
                        nc.vector.tensor_scalar(
                            a_t[:], w_b[:, j, :], zv[:, j, t:t + 1], 1.0,
                            ALU.mult, ALU.max,
                        )
                    q_t = sQ.tile([128, QL], SC_DT, tag="sq")
                    eng = nc.gpsimd if p2_gp else nc.vector
                    eng.tensor_tensor(q_t[:], a_t[:], adj_sb[:, t, :], ALU.mult)

                    for qh in range(2):
                        nc.tensor.matmul(
                            accs[qh][:], whS[:], q_t[:, qh * 512:(qh + 1) * 512],
                            start=(t == 0),
                            stop=(t == NT - 1 and not mode_act),
                        )
                        if mode_act:
                            nc.tensor.matmul(
                                accs[qh][:], whS[:],
                                adj_sb[:, t, qh * 512:(qh + 1) * 512],
                                start=False, stop=(t == NT - 1),
                            )
                # epilogue: divide by Z (row 64), elu, write into hTl
                for qh in range(2):
                    acc = accs[qh]
                    acc_sb = smalls.tile([65, 512], F32, tag="acc_sb")
                    nc.scalar.activation(acc_sb[:], acc[:], AF.Copy)
                    if dbgt and j == 0 and qh == 0:
                        nc.sync.dma_start(out=dbgt["d_acc0"][:, :], in_=acc_sb[:])
                    zrow = smalls.tile([1, 512], F32, tag="zrow")
                    nc.sync.dma_start(out=zrow[:], in_=acc_sb[64:65, :])
                    rz = smalls.tile([1, 512], F32, tag="rz")
                    nc.vector.reciprocal_approx_fast(rz[:], zrow[:])
                    zb = smalls.tile([64, 512], F32, tag="zb")
                    nc.gpsimd.partition_broadcast(zb[:], rz[:])
                    xv = smalls.tile([64, 512], F32, tag="xv")
                    nc.vector.tensor_tensor(xv[:], acc_sb[0:64, :], zb[:], ALU.mult)
                    if dbgt and j == 0 and qh == 0:
                        nc.sync.dma_start(out=dbgt["d_rz"][:, :], in_=rz[:])
                        nc.sync.dma_start(out=dbgt["d_zb"][:, :], in_=zb[:])
                        nc.sync.dma_start(out=dbgt["d_xv"][:, :], in_=xv[:])
                    mv = smalls.tile([64, 512], F32, tag="mv")
                    nc.vector.tensor_scalar_min(mv[:], xv[:], 0.0)
                    ev = smalls.tile([64, 512], F32, tag="ev")
                    nc.scalar.activation(ev[:], mv[:], AF.Exp)
                    rv = smalls.tile([64, 512], F32, tag="rv")
                    nc.vector.tensor_sub(rv[:], xv[:], mv[:])
                    if dbgt and j == 0 and qh == 0:
                        nc.sync.dma_start(out=dbgt["d_mv"][:, :], in_=mv[:])
                        nc.sync.dma_start(out=dbgt["d_ev"][:, :], in_=ev[:])
                        nc.sync.dma_start(out=dbgt["d_rv"][:, :], in_=rv[:])
                    hv = smalls.tile([64, 512], BF16, tag="hv")
                    nc.vector.affine_then_add(hv[:], rv[:], ev[:], scale=1.0, bias=-1.0)
                    nc.sync.dma_start(
                        out=hTl[(j % 2) * 64:(j % 2) * 64 + 64, j // 2,
                                qh * 512:(qh + 1) * 512],
                        in_=hv[:],
                    )

            if dbgt:
                nc.sync.dma_start(out=dbgt["d_zv"][:, :], in_=zv[:].rearrange("p a b -> p (a b)"))
                nc.sync.dma_start(out=dbgt["d_v2"][:, :], in_=v2[:].rearrange("p a b -> p (a b)"))

            # partial f1_2 (our feature block): row = w1out_g . hTl
            f12p = ps_f1.tile([128, QL], F32, tag="f1p", name="f12p")
            for t in range(2):
                for qh in range(2):
                    nc.tensor.matmul(
                        f12p[:, qh * 512:(qh + 1) * 512],
                        Wo1Rg_sb[:, t, :],
                        hTl[:, t, qh * 512:(qh + 1) * 512],
                        start=(t == 0), stop=(t == 1),
                    )
            nc.scalar.activation(prow[:], f12p[0:1, :], AF.Copy)

        # ---- stage C: AllGather hT (+ f1_2 partial row) ----
        nc.sync.dma_start(
            out=ag_in[0:256, :].rearrange("(t p) q -> p t q", p=128), in_=hTl[:]
        )
        nc.sync.dma_start(out=ag_in[256:257, :], in_=prow[:])
        nc.gpsimd.collective_compute(
            "AllGather",
            ALU.bypass,
            ins=[ag_in[:, :].opt()],
            outs=[ag_out[:, :, :].opt()],
            replica_groups=[list(range(NCORES))],
        )

        # ---- stage D: layer 2 ----
        wh2p = est.enter_context(tc.tile_pool(name="wh2p", bufs=1))
        w2bp = est.enter_context(tc.tile_pool(name="w2bp", bufs=1))
        with contextlib.ExitStack() as l2a:
            htag = l2a.enter_context(tc.tile_pool(name="htag", bufs=4))
            ps_w2 = l2a.enter_context(tc.tile_pool(name="ps_w2", bufs=2, space="PSUM"))
            ps_f2 = l2a.enter_context(tc.tile_pool(name="ps_f2", bufs=1, space="PSUM"))

            wh2_sb = wh2p.tile([128, NT, 42], F32)
            w2_b = w2bp.tile([128, QL], SC_DT)
            z2 = w2bp.tile([128, NT], F32)
            v22 = w2bp.tile([128, NT], F32)

            # Wh2Aug = h @ Wo  (node-major over all 4096 nodes)
            for sr in range(NS):
                hts = [htag.tile([128, QL], BF16, tag="htag", name=f"htag_{sr}_{i}") for i in range(KT)]
                for kf in range(KT):
                    nc.scalar.dma_start(
                        out=hts[kf][:],
                        in_=ag_out[(kf // 2) * 4 + sr,
                                   (kf % 2) * 128:(kf % 2) * 128 + 128, :],
                    )
                for mq in range(8):
                    mt = sr * 8 + mq
                    acc = ps_w2.tile([128, 42], F32)
                    for kf in range(KT):
                        nc.tensor.matmul(
                            acc[:], hts[kf][:, mq * 128:(mq + 1) * 128], Wo_sb[:, kf, :],
                            start=(kf == 0), stop=(kf == KT - 1),
                        )
                    nc.scalar.activation(wh2_sb[:, mt, :], acc[:], AF.Copy)
            nc.vector.memset(wh2_sb[:, :, 41:42], 1.0)
            nc.scalar.activation(z2[:], wh2_sb[:, :, 0], AF.Exp, scale=0.8)
            nc.scalar.activation(v22[:], wh2_sb[:, :, 0], AF.Exp, scale=0.2)

            # f1_2 for our q-slice: sum the two partner partial rows via WSel
            agr = w2bp.tile([NCORES, QL], BF16)
            nc.sync.dma_start(out=agr[:], in_=ag_out[:, 256, :])
            f2p = ps_f2.tile([128, QL], F32)
            for qh in range(2):
                nc.tensor.matmul(
                    f2p[:, qh * 512:(qh + 1) * 512],
                    WSel_sb[:],
                    agr[:, qh * 512:(qh + 1) * 512],
                    start=True, stop=True,
                )
            nc.scalar.activation(w2_b[:], f2p[:], AF.Exp, scale=0.8)
            if dbgt:
                nc.sync.dma_start(out=dbgt["d_wh2"][:, :], in_=wh2_sb[:].rearrange("p a b -> p (a b)"))

        with contextlib.ExitStack() as l2b:
            sA2 = l2b.enter_context(tc.tile_pool(name="sA2", bufs=2))
            sQ2 = l2b.enter_context(tc.tile_pool(name="sQ2", bufs=2))
            sW2 = l2b.enter_context(tc.tile_pool(name="sW2", bufs=3))
            ps_o2 = l2b.enter_context(tc.tile_pool(name="ps_o2", bufs=8, space="PSUM"))

            o2 = [ps_o2.tile([128, 42], F32, tag="o2", name=f"o2_{i}") for i in range(8)]
            for t in range(NT):
                wh2S = sW2.tile([128, 42], SC_DT, tag="wh2S")
                nc.vector.tensor_scalar_mul(wh2S[:], wh2_sb[:, t, :], v22[:, t:t + 1])
                a2 = sA2.tile([128, QL], SC_DT, tag="sa2")
                nc.vector.tensor_scalar(
                    a2[:], w2_b[:], z2[:, t:t + 1], 1.0, ALU.mult, ALU.max
                )
                q2 = sQ2.tile([128, QL], SC_DT, tag="sq2")
                eng = nc.gpsimd if (t % 4 == 3) else nc.vector
                eng.tensor_tensor(q2[:], a2[:], adj_sb[:, t, :], ALU.mult)
                for qi in range(8):
                    nc.tensor.matmul(
                        o2[qi][:], q2[:, qi * 128:(qi + 1) * 128], wh2S[:],
                        start=(t == 0), stop=(t == NT - 1),
                    )
            # epilogue: divide, elu, log_softmax (Exp phase, then Ln phase,
            # so the ACT table set switches at most once)
            keep = l2b.enter_context(tc.tile_pool(name="keep", bufs=8))
            h2s, nmxs, sss = [], [], []
            for qi in range(8):
                acc = o2[qi]
                rz = smalls.tile([128, 1], F32, tag="rz2")
                nc.vector.reciprocal(rz[:], acc[:, 41:42])
                x2 = smalls.tile([128, C], F32, tag="x2")
                nc.vector.tensor_scalar_mul(x2[:], acc[:, 1:41], rz[:])
                m2 = smalls.tile([128, C], F32, tag="m2")
                nc.vector.tensor_scalar_min(m2[:], x2[:], 0.0)
                e2 = smalls.tile([128, C], F32, tag="e2")
                nc.scalar.activation(e2[:], m2[:], AF.Exp)
                r2 = smalls.tile([128, C], F32, tag="r2")
                nc.vector.tensor_sub(r2[:], x2[:], m2[:])
                h2 = keep.tile([128, C], F32, tag="h2", name=f"h2_{qi}")
                nc.vector.affine_then_add(h2[:], r2[:], e2[:], scale=1.0, bias=-1.0)
                nmx = keep.tile([128, 1], F32, tag="nmx", name=f"nmx_{qi}")
                nc.vector.tensor_reduce(nmx[:], h2[:], AX.X, ALU.max, negate=True)
                es = smalls.tile([128, C], F32, tag="es")
                ss = keep.tile([128, 1], F32, tag="ss", name=f"ss_{qi}")
                nc.scalar.activation(es[:], h2[:], AF.Exp, bias=nmx[:], accum_out=ss[:])
                h2s.append(h2); nmxs.append(nmx); sss.append(ss)
            for qi in range(8):
                ls = smalls.tile([128, 1], F32, tag="ls")
                nc.scalar.activation(ls[:], sss[qi][:], AF.Ln)
                res = smalls.tile([128, C], F32, tag="res")
                nc.vector.tensor_scalar(
                    res[:], h2s[qi][:], nmxs[qi][:], ls[:], ALU.add, ALU.subtract
                )
                nc.sync.dma_start(out=out[qi * 128:(qi + 1) * 128, :], in_=res[:])


def _marshal(x, adj, W_heads, a_heads, W_out, a_out):
    xTf = np.ascontiguousarray(x.T)
    w1out = (W_out @ a_out[:C]).astype(np.float32)          # [512]
    w2out = (W_out @ a_out[C:]).astype(np.float32)          # [512]
    Wo_m = np.zeros((FIN, 42), np.float32)
    Wo_m[:, 0] = w2out
    Wo_m[:, 1:41] = W_out
    in_maps = []
    for c in range(NCORES):
        g, s = c // NS, c % NS
        heads = range(g * HG, (g + 1) * HG)
        Wa_m = np.zeros((FIN, HG * 66), np.float32)
        W1R_m = np.zeros((FIN, HG * 128), np.float32)
        for jj, h in enumerate(heads):
            Wa_m[:, jj * 66] = W_heads[h] @ a_heads[h, FH:]
            Wa_m[:, jj * 66 + 1: jj * 66 + 65] = W_heads[h]
            W1R_m[:, jj * 128:(jj + 1) * 128] = (W_heads[h] @ a_heads[h, :FH])[:, None]
        WSel_m = np.zeros((NCORES, 128), np.float32)
        for r in range(NCORES):
            if r % NS == s:
                WSel_m[r, :] = 1.0
        in_maps.append({
            "xT": xTf.astype(ml_dtypes.bfloat16),
            "xTq": np.ascontiguousarray(xTf[:, s * QL:(s + 1) * QL]).astype(ml_dtypes.bfloat16),
            "adjT": np.ascontiguousarray(adj[s * QL:(s + 1) * QL, :].T).astype(ADJ_NP),
            "Wa": Wa_m.astype(ml_dtypes.bfloat16),
            "W1R": W1R_m.astype(ml_dtypes.bfloat16),
            "Wo": Wo_m.astype(ml_dtypes.bfloat16),
            "Wo1Rg": np.ascontiguousarray(
                np.broadcast_to(w1out[g * 256:(g + 1) * 256, None], (256, 128))
            ).astype(ml_dtypes.bfloat16),
            "WSel": WSel_m.astype(ml_dtypes.bfloat16),
        })
    return in_maps


def kernel(**inputs):
    x = np.asarray(inputs["x"], np.float32)
    adj = np.asarray(inputs["adj"], np.float32)
    W_heads = np.asarray(inputs["W_heads"], np.float32)
    a_heads = np.asarray(inputs["a_heads"], np.float32)
    W_out = np.asarray(inputs["W_out"], np.float32)
    a_out = np.asarray(inputs["a_out"], np.float32)

    if "nc" not in _CACHE:
        _CACHE["nc"] = _build()
    nc = _CACHE["nc"]
    in_maps = _marshal(x, adj, W_heads, a_heads, W_out, a_out)
    res = run_bass_kernel_spmd(nc, in_maps, core_ids=list(range(NCORES)))
    full = np.empty((N, C), np.float32)
    for c in range(NS):  # g=0 copies carry the results
        full[c * QL:(c + 1) * QL] = res.results[c]["out"]
    return full
